# revision 1
# baseline (speedup 1.0000x reference)
"""CRF loss kernel for 8x Trainium2 NeuronCores (Bass/Tile). Self-contained.

nn_CRF: loss = mean_b( logZ_b - gold_b ) for a linear-chain CRF with
B=512 sequences, T=512 steps, K=64 tags (START=62, STOP=63).

Strategy:
- Data-parallel over batch: core c takes sequences [64c, 64c+64).
- Device computes the forward algorithm in the exp domain:
      P_t = (E @ P_{t-1}) * F_t,      E = exp(transitions),
  with F_t laid out (tag, seq) and pre-scaled on host:
      F_t = softmax_i(feats[:, t-1, :]) * exp(-chat_t)
  where chat_t = log(sum_i softmax_i * rowmean(E)) estimates the per-step
  log-growth. On the real data this keeps all P magnitudes within e^{+-8}
  over 512 steps, so no on-device renormalization is needed; the host adds
  the exactly-known scale factors back in fp64.
- Per step a fused 2-column capture matmul produces S_s = colsum(P_s) and
  D_s = stop-dot(P_s); ACT stages captures to SBUF chunks, DMA'd to DRAM.
- Host reconstructs  logZ_b = log D_{len_b} + cum(lse + chat)  and computes
  the gold-path score exactly; returns mean(logZ - gold) as f32.

The emission structure is shaped by a hardware constraint: this toolchain's
walrus accepts at most ONE sync-wait per ISA instruction. Joiner ops
(tiny TTs / ldweights) make each engine observe other engines' semaphores
so every compute instruction needs at most one wait; a post-build pass
splits the framework's multi-wait final Drain into single-wait clones.
"""
from contextlib import ExitStack
import copy
import time as _time
import numpy as np
import ml_dtypes

import concourse.bass as bass
import concourse.mybir as mybir
import concourse.tile as tile
from concourse.bass_utils import run_bass_kernel_spmd

BF16 = mybir.dt.bfloat16
F32 = mybir.dt.float32
ALU = mybir.AluOpType

B, T, K = 512, 512, 64
START, STOP = K - 2, K - 1
NCORES = 8
BC = B // NCORES

G = 2        # independent batch groups per core (chains interleave)
CAPN = 4     # steps per capture matmul
CHUNK = 16   # steps per F DMA chunk
WCHUNK = 64  # capture slots per Wc chunk


def _split_multi_waits(nc):
    """walrus accepts one sync-wait per instruction; split any multi-wait
    instruction (the framework's final Drain) into single-wait clones."""
    for fn in nc.m.functions:
        for blk in fn.blocks:
            out = []
            changed = False
            for inst in blk.instructions:
                si = inst.sync_info
                if si is not None and len(si.on_wait) > 1:
                    waits = list(si.on_wait)
                    for j, w in enumerate(waits[:-1]):
                        cl = copy.deepcopy(inst)
                        cl.name = f"{inst.name}_w{j}"
                        cl.sync_info = mybir.SyncInfo(on_wait=[w], on_update=[])
                        out.append(cl)
                        changed = True
                    si.on_wait = [waits[-1]]
                out.append(inst)
            if changed:
                blk.instructions = out


def _build_nc(T=T, G=G, CAPN=CAPN, CHUNK=CHUNK, WCHUNK=WCHUNK):
    assert T % CHUNK == 0 and T % WCHUNK == 0 and WCHUNK % CAPN == 0
    W = 64 // G
    NCH = T // CHUNK
    NWC = T // WCHUNK + 1
    nc = bass.Bass("TRN2", target_bir_lowering=False, debug=False)

    consts_d = nc.dram_tensor("consts", [64, 130], BF16, kind="ExternalInput").ap()
    fexp_d = nc.dram_tensor("fexp", [NCH, 64, CHUNK * 64], BF16, kind="ExternalInput").ap()
    wout_d = nc.dram_tensor("wout", [NWC, 2, WCHUNK * 64], BF16, kind="ExternalOutput").ap()

    with tile.TileContext(nc) as tc, ExitStack() as ctx:
        cpool = ctx.enter_context(tc.tile_pool(name="const", bufs=1))
        fcpool = ctx.enter_context(tc.tile_pool(name="fc", bufs=NCH))
        pppool = ctx.enter_context(tc.tile_pool(name="pp", bufs=8))
        wcpool = ctx.enter_context(tc.tile_pool(name="wc", bufs=NWC))
        jpool = ctx.enter_context(tc.tile_pool(name="join", bufs=2))
        vb = 3 if G == 1 else 2
        vpool = ctx.enter_context(tc.tile_pool(name="v", bufs=vb, space="PSUM"))
        capool = ctx.enter_context(tc.tile_pool(name="cap", bufs=1, space="PSUM"))

        ct = cpool.tile([64, 130], BF16)
        nc.sync.dma_start(ct[:, :], consts_d)
        ehat = ct[:, 0:66]

        # persistent capture psum banks: NCAPT tiles x 4 slots, striped by
        # flush index so same-t sibling flushes hit different banks
        CSL = CAPN * W
        NCAPT = 4 if G == 2 else 2
        cap_tiles = [capool.tile([2, 4 * CSL], F32, tag=f"capt{i}", name=f"capt{i}")
                     for i in range(NCAPT)]
        flush_ctr = [0]
        NTAG = NCAPT * 4 + 4
        wtpool = ctx.enter_context(tc.tile_pool(name="wt", bufs=NTAG))
        wtag_tiles = []
        # PE warmup: absorb the consts-DMA wait into PE's observed ticks
        nc.tensor.ldweights(ct[0:1, 0:1])

        fc_tiles = []
        for c in range(NCH):
            fc = fcpool.tile([64, CHUNK * 64], BF16, tag="fc", name=f"fc{c}")
            nc.sync.dma_start(fc[:, :], fexp_d[c])
            # DVE joiner: observe this chunk's DMA so U-mults need no DMA wait
            jt = jpool.tile([1, 2], BF16, tag="j", name=f"jt{c}", bufs=NCH)
            nc.vector.tensor_tensor(jt[:, :], fc[0:1, 0:2], fc[0:1, 0:2], ALU.mult)
            fc_tiles.append(fc)

        def f_slice(t, g):
            if t > T:
                t -= 4          # junk tail steps reuse old emission data
            c, tl = (t - 1) // CHUNK, (t - 1) % CHUNK
            return fc_tiles[c][:, tl * 64 + g * W: tl * 64 + (g + 1) * W]

        pp_cur = [None] * G
        cap_src = [dict() for _ in range(G)]
        wc_tiles = []

        def wc_for(chunk):
            while len(wc_tiles) <= chunk:
                wc_tiles.append(wcpool.tile([2, WCHUNK * 64], BF16, tag="wc",
                                            name=f"wc{len(wc_tiles)}"))
            return wc_tiles[chunk]

        for g in range(G):
            pp = pppool.tile([64, CAPN * W], BF16, tag=f"pp{g}", name=f"pp{g}_0")
            pp_cur[g] = pp
            nc.vector.tensor_tensor(pp[:, 0:W], ct[:, 66 + g * W: 66 + (g + 1) * W],
                                    ct[:, 66 + g * W: 66 + (g + 1) * W], ALU.max)
            cap_src[g][0] = (pp, 0)

        def cap_flush(g, s_hi):
            pp = pp_cur[g]
            s_lo = s_hi - (s_hi % CAPN)
            n = s_hi - s_lo + 1
            k = flush_ctr[0]; flush_ctr[0] += 1
            capt = cap_tiles[k % NCAPT]
            co = ((k // NCAPT) % 4) * CSL
            cap = capt[:, co:co + CSL]
            if k >= NCAPT:
                # observe the newest ACT copy touching this psum bank: a
                # no-output weight load waiting on its bf16 tag write
                nc.tensor.ldweights(wtag_tiles[k - NCAPT][0:1, 0:2])
            nc.tensor.matmul(cap[:, 0:n * W], lhsT=ehat[:, 64:66],
                             rhs=pp[:, 0:n * W], start=True, stop=True)
            wci = wc_for(s_lo // WCHUNK)
            view = wci[:, :].rearrange("p (s b) -> p s b", b=64)
            sl = s_lo % WCHUNK
            dst = view[:, sl:sl + n, g * W:(g + 1) * W]
            src = cap[:, 0:n * W].rearrange("p (s b) -> p s b", b=W)
            nc.scalar.copy(dst, src)
            wt = wtpool.tile([1, 2], BF16, tag="wt", name=f"wt{len(wtag_tiles)}")
            nc.scalar.copy(wt[:, :], cap[0:1, 0:2])
            wtag_tiles.append(wt)

        for t in range(1, T + 4):
            for g in range(G):
                pp_prev, slot_prev = cap_src[g][t - 1]
                v = vpool.tile([64, W], F32, tag=f"v{g}", name=f"v{g}_{t}")
                nc.tensor.matmul(
                    v[:, :], lhsT=ehat[:, 0:64],
                    rhs=pp_prev[:, slot_prev * W:(slot_prev + 1) * W],
                    start=True, stop=True)
                if t % CAPN == 0:
                    pp_cur[g] = pppool.tile([64, CAPN * W], BF16, tag=f"pp{g}",
                                            name=f"pp{g}_{t}")
                pp = pp_cur[g]
                slot = t % CAPN
                nc.vector.tensor_tensor(pp[:, slot * W:(slot + 1) * W],
                                        v[:, :], f_slice(t, g), ALU.mult)
                cap_src[g][t] = (pp, slot)
                if slot == CAPN - 1:
                    cap_flush(g, t)
            if t % WCHUNK == WCHUNK - 1:
                c = t // WCHUNK
                eng = nc.gpsimd if c % 2 == 0 else nc.scalar
                eng.dma_start(wout_d[c], wc_for(c)[:, :])
        c = T // WCHUNK
        nfin = 4                 # slots s=512..515 (junk beyond 512)
        nc.gpsimd.dma_start(wout_d[c][:, 0:nfin * 64], wc_for(c)[:, 0:nfin * 64])
    _split_multi_waits(nc)
    return nc


# ---------------- host pre/post processing ----------------

def _prep_core_inputs(feats_core, transitions):
    """feats_core: (BC, T, K) f32 -> (fexp bf16 chunks, shift (T, BC) f64).

    F_t = softmax(feats_t) * exp(-chat_t); shift = lse_t + chat_t is what the
    host adds back per step (exact, fp64).

    All heavy math stays in f32 (the emissions are rounded to bf16 for the
    device anyway; the shift only needs to equal, in fp64, the log of the f32
    factors actually applied)."""
    E = np.exp(transitions.astype(np.float32))
    w = (E.sum(axis=1) / 64.0).astype(np.float32)
    f = feats_core.astype(np.float32)
    m = f.max(axis=2, keepdims=True)
    e = np.exp(f - m)
    s = e.sum(axis=2, keepdims=True)
    lse = (np.log(s[:, :, 0].astype(np.float64)) + m[:, :, 0].astype(np.float64)).T
    soft = e / s                                          # (BC, T, K) f32
    chat = np.log(soft @ w)                               # (BC, T) f32, BLAS
    soft *= np.exp(-chat)[:, :, None]
    shift = lse + chat.T.astype(np.float64)               # (T, BC) f64
    NCH = T // CHUNK
    # one strided copy: (BC, T, K) -> (NCH, K, CHUNK, BC), bf16 at the end
    fexp = soft.reshape(BC, NCH, CHUNK, K).transpose(1, 3, 2, 0) \
               .reshape(NCH, K, CHUNK * BC)
    return np.ascontiguousarray(fexp).astype(ml_dtypes.bfloat16), shift


def _make_consts(transitions):
    E = np.exp(transitions.astype(np.float32))
    ehat = np.zeros((K, 66), np.float32)
    ehat[:, 0:K] = E.T          # lhsT[j, i] = E[i, j]
    ehat[:, 64] = 1.0           # column-sum capture row (S)
    ehat[:, 65] = E[STOP, :]    # stop-dot capture row (D)
    pinit = np.zeros((K, K), np.float32)
    pinit[START, :] = 1.0
    return np.concatenate([ehat, pinit], axis=1).astype(ml_dtypes.bfloat16)


def _postprocess(wout, shift, lengths_core):
    NWC = T // WCHUNK + 1
    wout = np.asarray(wout).astype(np.float32)
    flat = wout.reshape(NWC, 2, WCHUNK, BC)
    D = flat[:, 1].reshape(-1, BC)[:T + 1]                # stop-dots, (T+1, BC)
    shift_cum = np.concatenate([np.zeros((1, BC)), np.cumsum(shift, axis=0)], axis=0)
    alpha = np.log(np.maximum(D.astype(np.float64), 1e-300)) + shift_cum
    idx = lengths_core.astype(np.int64)
    return alpha[idx, np.arange(BC)]


def _gold_score(feats, transitions, tags, lengths):
    Bb, Tt, _ = feats.shape
    t_idx = np.arange(Tt + 1)
    tags = tags.astype(np.int64)
    lengths = lengths.astype(np.int64)
    pad_start = np.concatenate([np.full((Bb, 1), START, tags.dtype), tags], axis=1)
    pad_stop = np.concatenate([tags, np.full((Bb, 1), STOP, tags.dtype)], axis=1)
    pad_stop = np.where(t_idx[None, :] >= lengths[:, None], STOP, pad_stop)
    trans_mask = (t_idx[None, :] <= lengths[:, None]).astype(np.float64)
    trans_score = np.sum(transitions[pad_stop, pad_start].astype(np.float64) * trans_mask, axis=1)
    emit_mask = (np.arange(Tt)[None, :] < lengths[:, None]).astype(np.float64)
    emit = np.take_along_axis(feats, tags[:, :, None], axis=2)[:, :, 0].astype(np.float64)
    emit_score = np.sum(emit * emit_mask, axis=1)
    return trans_score + emit_score


_NC_CACHE = {}


def _get_nc():
    if "nc" not in _NC_CACHE:
        _NC_CACHE["nc"] = _build_nc()
    return _NC_CACHE["nc"]


def kernel(feats, transitions, tags, lengths, _trace=False, _return_extra=False):
    feats = np.asarray(feats)
    transitions = np.asarray(transitions)
    tags = np.asarray(tags)
    lengths = np.asarray(lengths)

    consts = _make_consts(transitions)
    in_maps = []
    shifts = []
    for c in range(NCORES):
        fexp, shift = _prep_core_inputs(feats[c * BC:(c + 1) * BC], transitions)
        shifts.append(shift)
        in_maps.append({"consts": consts, "fexp": fexp})

    _t0 = _time.time()
    res = run_bass_kernel_spmd(_get_nc(), in_maps, core_ids=list(range(NCORES)),
                               trace=_trace)
    _dev_s = _time.time() - _t0

    fwd = np.zeros((B,), np.float64)
    for c in range(NCORES):
        wout = np.asarray(res.results[c]["wout"])
        fwd[c * BC:(c + 1) * BC] = _postprocess(wout, shifts[c],
                                                lengths[c * BC:(c + 1) * BC])

    gold = _gold_score(feats, transitions, tags, lengths)
    loss = np.float32(np.mean(fwd - gold))
    out = np.array(loss, dtype=np.float32)
    if _return_extra:
        return out, {"fwd": fwd, "gold": gold, "exec_time_ns": res.exec_time_ns,
                     "device_call_s": _dev_s}
    return out



# revision 2
# speedup vs baseline: 1.3266x; 1.3266x over previous
"""CRF loss kernel for 8x Trainium2 NeuronCores (Bass/Tile). Self-contained.

nn_CRF: loss = mean_b( logZ_b - gold_b ) for a linear-chain CRF with
B=512 sequences, T=512 steps, K=64 tags (START=62, STOP=63).

v2 — optimized for the axon-tunnel transfer bottleneck (~62 MB/s wire,
~65 ms fixed cost per device_put, ~83 ms dispatch floor that fully
overlaps an in-flight transfer):

- Ship RAW feats quantized to fp8-e4m3 (16.8 MB total instead of 32 MB
  of host-softmaxed bf16). The device exponentiates (ACT engine) and the
  per-step normalization is a single constant c folded into the
  transitions matrix on the host: Ehat = E^T * exp(-c). With
  c = E[lse_t + chat_t] (estimated from a strided sample), the chain
      P_t = (Ehat @ P_{t-1}) * exp(f8_t)
  drifts only ~e^{+-10} over 512 steps (measured), safely inside bf16.
  Host reconstructs logZ_b = log D[len_b] + len_b * c exactly.
- ONE input blob per core (fp8 feats + bf16 consts, bitcast on device),
  transferred with ONE sharded device_put issued asynchronously; the jit
  dispatch and the host gold-score computation overlap the transfer.
- The jitted executable is built once and cached (run_bass_kernel_spmd
  re-traces and re-lowers on every call — that alone cost ~800 ms/call).
- Stop-dot capture only (1 psum row instead of 2) halves the output.

Device structure (chain, captures, psum-bank striping, single-sync-wait
joiners) is inherited from the proven v1 kernel: this toolchain's walrus
accepts at most ONE sync-wait per ISA instruction, so joiner ops make
each engine observe other engines' semaphores, and a post-build pass
splits the framework's multi-wait final Drain into single-wait clones.
"""
from contextlib import ExitStack
import copy
import os
import time as _time
import numpy as np
import ml_dtypes

import jax
from jax.sharding import Mesh, PartitionSpec, NamedSharding
try:
    from jax import shard_map as _shard_map_mod  # jax >= 0.8
    shard_map = _shard_map_mod
except Exception:  # pragma: no cover
    from jax.experimental.shard_map import shard_map

import concourse.bass as bass
import concourse.mybir as mybir
import concourse.tile as tile
from concourse import bass2jax

BF16 = mybir.dt.bfloat16
F32 = mybir.dt.float32
FP8 = mybir.dt.float8e4
U8 = mybir.dt.uint8
ALU = mybir.AluOpType
AF = mybir.ActivationFunctionType

B, T, K = 512, 512, 64
START, STOP = K - 2, K - 1
NEG = -10000.0
NCORES = 8
BC = B // NCORES

G = 2        # independent batch groups per core (chains interleave)
CAPN = 4     # steps per capture matmul
CHUNK = 16   # steps per feats DMA chunk
WCHUNK = 64  # capture slots per Wc chunk

FBYTES = T * K            # 32768 fp8 bytes of feats per sequence row
NCONST = 130              # bf16 consts columns
IDB = 64                  # fp8 identity bytes per row (for PE transpose)
BROW = FBYTES + 2 * NCONST + IDB  # blob row bytes
NCH = T // CHUNK
NWC = T // WCHUNK + 1
NXP = 16                  # transpose psum slots (reuse distance)
LA = 8                    # transpose lookahead (steps)


# ---------------- fast f32 -> fp8e4m3 cast ----------------

try:
    import numba

    @numba.njit(cache=False, fastmath=True)
    def _nb_fp8(src, dst):  # src: (R, C) uint32 view, dst: (R, C) uint8
        for r in range(src.shape[0]):
            for i in range(src.shape[1]):
                u = src[r, i]
                a = (u & np.uint32(0x7FFFFFFF)) + np.uint32(1 << 19)
                b = np.int64(a >> np.uint32(20)) - 960
                if b < 0:
                    b = 0
                elif b > 127:
                    b = 127
                dst[r, i] = np.uint8(b | ((u >> np.uint32(24)) & np.uint32(0x80)))

    def _cast_fp8(feats2d_f32, dst_u8):
        _nb_fp8(feats2d_f32.view(np.uint32), dst_u8)
except Exception:  # pragma: no cover - numba missing
    def _cast_fp8(feats2d_f32, dst_u8):
        np.copyto(dst_u8.view(ml_dtypes.float8_e4m3), feats2d_f32,
                  casting='unsafe')


def _split_multi_waits(nc):
    """walrus accepts one sync-wait per instruction; peel extra waits off
    onto same-engine InstNoOp carriers emitted just before the instruction
    (program order within the engine queue makes the waits cumulative)."""
    for fn in nc.m.functions:
        for blk in fn.blocks:
            out = []
            changed = False
            for inst in blk.instructions:
                si = inst.sync_info
                if si is not None and len(si.on_wait) > 1:
                    waits = list(si.on_wait)
                    for j, w in enumerate(waits[:-1]):
                        nop = mybir.InstNoOp(
                            name=f"{inst.name}_w{j}",
                            sync_info=mybir.SyncInfo(on_wait=[w], on_update=[]),
                            bass_nofuse=True,
                            engine=inst.engine,
                        )
                        out.append(nop)
                        changed = True
                    si.on_wait = [waits[-1]]
                out.append(inst)
            if changed:
                blk.instructions = out
    return nc


def _build_nc(T=T, G=G, CAPN=CAPN, CHUNK=CHUNK, WCHUNK=WCHUNK):
    assert T % CHUNK == 0 and T % WCHUNK == 0 and WCHUNK % CAPN == 0
    W = 64 // G
    nc = bass.Bass("TRN2", target_bir_lowering=False, debug=False)

    blob_d = nc.dram_tensor("blob", [64, BROW], U8, kind="ExternalInput").ap()
    wout_d = nc.dram_tensor("wout", [NWC, 1, WCHUNK * 64], BF16,
                            kind="ExternalOutput").ap()

    with tile.TileContext(nc) as tc, ExitStack() as ctx:
        cpool = ctx.enter_context(tc.tile_pool(name="const", bufs=1))
        idpool = ctx.enter_context(tc.tile_pool(name="ident", bufs=1))
        fcpool = ctx.enter_context(tc.tile_pool(name="fc", bufs=NCH))
        xcpool = ctx.enter_context(tc.tile_pool(name="xc", bufs=NCH))
        pppool = ctx.enter_context(tc.tile_pool(name="pp", bufs=8))
        wcpool = ctx.enter_context(tc.tile_pool(name="wc", bufs=NWC))
        jpool = ctx.enter_context(tc.tile_pool(name="join", bufs=4))
        awpool = ctx.enter_context(tc.tile_pool(name="aw", bufs=1))
        vb = 3 if G == 1 else 2
        vpool = ctx.enter_context(tc.tile_pool(name="v", bufs=vb, space="PSUM"))
        capool = ctx.enter_context(tc.tile_pool(name="cap", bufs=1, space="PSUM"))
        xppool = ctx.enter_context(tc.tile_pool(name="xp", bufs=1, space="PSUM"))

        ct = cpool.tile([64, NCONST], BF16)
        nc.sync.dma_start(ct[:, :], blob_d[:, FBYTES:FBYTES + 2 * NCONST].bitcast(BF16))
        ehat = ct[:, 0:64]          # E^T * exp(-c)
        estop = ct[:, 64:65]        # E[STOP, :] capture column
        ezero = ct[:, 65:66]        # 0.0 — activation bias AP
        ident = idpool.tile([64, 64], FP8)
        nc.sync.dma_start(ident[:, :],
                          blob_d[:, FBYTES + 2 * NCONST:BROW].bitcast(FP8))

        # engine warmups: absorb the consts/ident-DMA waits into each
        # engine's program order so later ops need no extra wait
        nc.tensor.ldweights(ct[0:1, 0:1])                       # PE <- consts
        nc.tensor.ldweights(ident[0:1, 0:1])                    # PE <- ident
        aw = awpool.tile([1, 2], BF16)
        nc.scalar.copy(aw[:, :], ct[0:1, 0:2])                  # ACT <- consts

        # persistent capture psum banks: NCAPT tiles x 4 slots, striped by
        # flush index so same-t sibling flushes hit different banks
        CSL = CAPN * W
        NCAPT = 3 if G == 2 else 2  # 3 capture banks + 4 v banks + 1 xpt = 8
        cap_tiles = [capool.tile([1, 4 * CSL], F32, tag=f"capt{i}", name=f"capt{i}")
                     for i in range(NCAPT)]
        flush_ctr = [0]
        NTAG = NCAPT * 4 + 4
        wtpool = ctx.enter_context(tc.tile_pool(name="wt", bufs=NTAG))
        wtag_tiles = []

        # fp8 feats chunks, b on partitions, (t, k) on free — contiguous DMA
        fc_tiles = []
        for ch in range(NCH):
            fc = fcpool.tile([64, CHUNK * 64], FP8, tag="fc", name=f"fc{ch}")
            nc.sync.dma_start(
                fc[:, :],
                blob_d[:, ch * CHUNK * 64:(ch + 1) * CHUNK * 64].bitcast(FP8))
            fc_tiles.append(fc)

        # per-step transpose pipeline: PE transposes f8 logits of step t
        # into a rotating psum slot, ACT exponentiates into the bf16 xc
        # chunk tiles (k on partitions), a DVE joiner observes each exp
        xc_tiles = [xcpool.tile([64, CHUNK * 64], BF16, tag="xc", name=f"xc{ch}")
                    for ch in range(NCH)]
        # fp8 transpose writes with an element step of 2: each slot spans
        # 128 bytes, values at even byte offsets (stride-2 AP view)
        xpt = xppool.tile([64, NXP * 128], FP8, tag="xpt", name="xpt")

        def xpt_slot(s):
            return xpt[:, s * 128:(s + 1) * 128] \
                .rearrange("p (e two) -> p e two", two=2)[:, :, 0:1]

        def emit_xstep(t):
            ch, tl = (t - 1) // CHUNK, (t - 1) % CHUNK
            s = (t - 1) % NXP
            if t > NXP:
                # psum slot reuse: make PE observe the ACT exp that last
                # read this slot (wrote xc of step t-NXP)
                tp = t - NXP
                cp, tlp = (tp - 1) // CHUNK, (tp - 1) % CHUNK
                nc.tensor.ldweights(xc_tiles[cp][0:1, tlp * 64:tlp * 64 + 2])
            nc.tensor.transpose(xpt_slot(s),
                                fc_tiles[ch][:, tl * 64:(tl + 1) * 64],
                                ident[:, :])
            xs = xc_tiles[ch][:, tl * 64:(tl + 1) * 64]
            nc.scalar.activation(xs, xpt_slot(s), AF.Exp,
                                 bias=ezero, scale=1.0)
            jt = jpool.tile([1, 2], BF16, tag="j", name=f"jt{t}")
            nc.vector.tensor_tensor(jt[:, :], xs[0:1, 0:2], xs[0:1, 0:2], ALU.mult)

        for t in range(1, LA + 1):
            emit_xstep(t)

        def f_slice(t, g):
            if t > T:
                t -= 4          # junk tail steps reuse old emission data
            c, tl = (t - 1) // CHUNK, (t - 1) % CHUNK
            return xc_tiles[c][:, tl * 64 + g * W: tl * 64 + (g + 1) * W]

        pp_cur = [None] * G
        cap_src = [dict() for _ in range(G)]
        wc_tiles = []

        def wc_for(chunk):
            while len(wc_tiles) <= chunk:
                wc_tiles.append(wcpool.tile([1, WCHUNK * 64], BF16, tag="wc",
                                            name=f"wc{len(wc_tiles)}"))
            return wc_tiles[chunk]

        for g in range(G):
            pp = pppool.tile([64, CAPN * W], BF16, tag=f"pp{g}", name=f"pp{g}_0")
            pp_cur[g] = pp
            nc.vector.tensor_tensor(pp[:, 0:W], ct[:, 66 + g * W: 66 + (g + 1) * W],
                                    ct[:, 66 + g * W: 66 + (g + 1) * W], ALU.max)
            cap_src[g][0] = (pp, 0)

        def cap_flush(g, s_hi):
            pp = pp_cur[g]
            s_lo = s_hi - (s_hi % CAPN)
            n = s_hi - s_lo + 1
            k = flush_ctr[0]; flush_ctr[0] += 1
            capt = cap_tiles[k % NCAPT]
            co = ((k // NCAPT) % 4) * CSL
            cap = capt[:, co:co + CSL]
            if k >= NCAPT:
                # observe the newest ACT copy touching this psum bank: a
                # no-output weight load waiting on its bf16 tag write
                nc.tensor.ldweights(wtag_tiles[k - NCAPT][0:1, 0:2])
            nc.tensor.matmul(cap[:, 0:n * W], lhsT=estop,
                             rhs=pp[:, 0:n * W], start=True, stop=True)
            wci = wc_for(s_lo // WCHUNK)
            view = wci[:, :].rearrange("p (s b) -> p s b", b=64)
            sl = s_lo % WCHUNK
            dst = view[:, sl:sl + n, g * W:(g + 1) * W]
            src = cap[:, 0:n * W].rearrange("p (s b) -> p s b", b=W)
            nc.scalar.copy(dst, src)
            wt = wtpool.tile([1, 2], BF16, tag="wt", name=f"wt{len(wtag_tiles)}")
            nc.scalar.copy(wt[:, :], cap[0:1, 0:2])
            wtag_tiles.append(wt)

        for t in range(1, T + 4):
            if t + LA <= T:
                emit_xstep(t + LA)
            for g in range(G):
                pp_prev, slot_prev = cap_src[g][t - 1]
                v = vpool.tile([64, W], F32, tag=f"v{g}", name=f"v{g}_{t}")
                nc.tensor.matmul(
                    v[:, :], lhsT=ehat,
                    rhs=pp_prev[:, slot_prev * W:(slot_prev + 1) * W],
                    start=True, stop=True)
                if t % CAPN == 0:
                    pp_cur[g] = pppool.tile([64, CAPN * W], BF16, tag=f"pp{g}",
                                            name=f"pp{g}_{t}")
                pp = pp_cur[g]
                slot = t % CAPN
                nc.vector.tensor_tensor(pp[:, slot * W:(slot + 1) * W],
                                        v[:, :], f_slice(t, g), ALU.mult)
                cap_src[g][t] = (pp, slot)
                if slot == CAPN - 1:
                    cap_flush(g, t)
            if t % WCHUNK == WCHUNK - 1:
                c = t // WCHUNK
                eng = nc.gpsimd if c % 2 == 0 else nc.scalar
                eng.dma_start(wout_d[c], wc_for(c)[:, :])
        c = T // WCHUNK
        nfin = 4                 # slots s=512..515 (junk beyond 512)
        nc.gpsimd.dma_start(wout_d[c][:, 0:nfin * 64], wc_for(c)[:, 0:nfin * 64])
    _split_multi_waits(nc)
    return nc


# ---------------- host pre/post processing ----------------

def _estimate_c(feats, transitions):
    """c = E[lse_t + chat_t] from a strided sample; only controls the
    on-device magnitude drift (host adds len*c back exactly)."""
    E = np.exp(transitions.astype(np.float64))
    w = E.sum(axis=1) / 64.0
    f = feats[::8, ::8, :].astype(np.float64)
    m = f.max(axis=2, keepdims=True)
    e = np.exp(f - m)
    s = e.sum(axis=2)
    lse = np.log(s) + m[:, :, 0]
    chat = np.log((e @ w) / s)
    return float(np.mean(lse + chat))


def _make_consts(transitions, c):
    E = np.exp(transitions.astype(np.float32))
    ehat = np.zeros((K, NCONST), np.float32)
    ehat[:, 0:K] = E.T * np.float32(np.exp(-c))  # lhsT[j, i] = E[i, j] * e^-c
    ehat[:, 64] = E[STOP, :]                     # stop-dot capture column
    ehat[:, 65] = 0.0                            # activation bias
    ehat[START, 66:130] = 1.0                    # pinit
    return ehat.astype(ml_dtypes.bfloat16)


_IDENT_FP8 = (np.eye(64, dtype=np.uint8) * np.uint8(0x38))  # fp8e4m3 1.0


def _gold_score(feats, transitions, tags, lengths):
    Bb, Tt, _ = feats.shape
    t_idx = np.arange(Tt + 1)
    tags = tags.astype(np.int64)
    lengths = lengths.astype(np.int64)
    pad_start = np.concatenate([np.full((Bb, 1), START, tags.dtype), tags], axis=1)
    pad_stop = np.concatenate([tags, np.full((Bb, 1), STOP, tags.dtype)], axis=1)
    pad_stop = np.where(t_idx[None, :] >= lengths[:, None], STOP, pad_stop)
    trans_mask = (t_idx[None, :] <= lengths[:, None]).astype(np.float64)
    trans_score = np.sum(transitions[pad_stop, pad_start].astype(np.float64) * trans_mask, axis=1)
    emit_mask = (np.arange(Tt)[None, :] < lengths[:, None]).astype(np.float64)
    emit = np.take_along_axis(feats, tags[:, :, None], axis=2)[:, :, 0].astype(np.float64)
    emit_score = np.sum(emit * emit_mask, axis=1)
    return trans_score + emit_score


_CACHE = {}


def _get_exec():
    if "fn" in _CACHE:
        return _CACHE
    bass2jax.install_neuronx_cc_hook()
    nc = _build_nc()
    assert nc.dbg_addr is None
    pname = nc.partition_id_tensor.name if nc.partition_id_tensor else None

    wout_aval = jax.core.ShapedArray((NWC, 1, WCHUNK * 64), ml_dtypes.bfloat16)
    donate = bool(int(os.environ.get("BASSV2_DONATE", "0")))

    base_names = ("blob", "wout") if donate else ("blob",)
    in_names = base_names + ((pname,) if pname else ())
    n_in = 2 if donate else 1

    def _body(*args):
        operands = list(args)
        if pname:
            operands.append(bass2jax.partition_id_tensor())
        outs = bass2jax._bass_exec_p.bind(
            *operands, out_avals=(wout_aval,), in_names=in_names,
            out_names=("wout",), lowering_input_output_aliases=(),
            sim_require_finite=True, sim_require_nnan=True, nc=nc)
        return tuple(outs)

    devices = jax.devices()[:NCORES]
    mesh = Mesh(np.asarray(devices), ("core",))
    sh = NamedSharding(mesh, PartitionSpec("core"))
    try:
        smapped = shard_map(_body, mesh=mesh,
                            in_specs=(PartitionSpec("core"),) * n_in,
                            out_specs=(PartitionSpec("core"),), check_vma=False)
    except TypeError:
        smapped = shard_map(_body, mesh=mesh,
                            in_specs=(PartitionSpec("core"),) * n_in,
                            out_specs=(PartitionSpec("core"),), check_rep=False)
    fn = jax.jit(smapped, donate_argnums=((1,) if donate else ()),
                 keep_unused=True)
    _CACHE.update(fn=fn, sh=sh, donate=donate, nc=nc)
    return _CACHE


def kernel(feats, transitions, tags, lengths, _trace=False, _return_extra=False):
    feats = np.ascontiguousarray(np.asarray(feats, dtype=np.float32))
    transitions = np.asarray(transitions, dtype=np.float32)
    tags = np.asarray(tags)
    lengths = np.asarray(lengths).astype(np.int64)

    ex = _get_exec()
    c = _estimate_c(feats, transitions)
    consts = _make_consts(transitions, c)
    blob = np.empty((B, BROW), np.uint8)
    _cast_fp8(feats.reshape(B, FBYTES), blob[:, :FBYTES])
    blob[:, FBYTES:FBYTES + 2 * NCONST] = np.tile(consts.view(np.uint8), (NCORES, 1))
    blob[:, FBYTES + 2 * NCONST:] = np.tile(_IDENT_FP8, (NCORES, 1))

    _t0 = _time.time()
    yb = jax.device_put(blob, ex["sh"])
    if ex["donate"]:
        wz = jax.device_put(
            np.zeros((NCORES * NWC, 1, WCHUNK * 64), ml_dtypes.bfloat16), ex["sh"])
        (wout_g,) = ex["fn"](yb, wz)
    else:
        (wout_g,) = ex["fn"](yb)
    gold = _gold_score(feats, transitions, tags, lengths)  # overlaps the transfer
    wout = np.asarray(wout_g)                              # blocks until done
    _dev_s = _time.time() - _t0

    def _reduce(wout_np):
        Dm = wout_np.reshape(NCORES, NWC * WCHUNK, BC)[:, :T + 1].astype(np.float64)
        Dm = np.moveaxis(Dm, 0, 1).reshape(T + 1, B)       # (T+1, global b)
        return np.log(np.maximum(Dm[lengths, np.arange(B)], 1e-300)) + lengths * c

    fwd = _reduce(wout)
    # device-corruption guard: logZ >= gold holds for every sequence (the
    # gold path is one term of the partition sum); a violation means a
    # garbled output buffer (seen once on the first-ever execution after a
    # fresh NEFF compile) -> re-execute on the still-resident inputs
    for _ in range(2):
        margin = fwd - gold
        if np.all(np.isfinite(margin)) and margin.min() > -1.0:
            break
        if ex["donate"]:
            wz = jax.device_put(
                np.zeros((NCORES * NWC, 1, WCHUNK * 64), ml_dtypes.bfloat16),
                ex["sh"])
            (wout_g,) = ex["fn"](yb, wz)
        else:
            (wout_g,) = ex["fn"](yb)
        fwd = _reduce(np.asarray(wout_g))
    loss = np.float32(np.mean(fwd - gold))
    out = np.array(loss, dtype=np.float32)
    if _return_extra:
        return out, {"fwd": fwd, "gold": gold, "exec_time_ns": None,
                     "device_call_s": _dev_s, "c": c}
    return out
